# revision 1
# baseline (speedup 1.0000x reference)
"""Trainium2 Bass kernel for nn_KernelMachine (random Fourier features).

out[n,m] = sum_f sqrt(2/F) * cos(x_n . a_f + b_f) * W[f*M+m]

Data-parallel over 8 NeuronCores (N sharded, a/b/W replicated).

Per core (N_loc=4096, D=16, F=4096, M=16), pipeline per tile
(tile = one f-chunk of 128 x one n-group of 1024, 128 tiles):

  m1   (PE):  t = (x @ a.T + b') / 2pi in PSUM fp32.  bf16 operands with
              2-limb x + 1-limb a (K=34 = xh/xl rows + bh/bl bias rows);
              angle error ~5e-3 rad, well inside tolerance.  The two
              512-col halves use row-groups 0 and 64 (stationary + moving
              duplicated there) so they run as concurrent row tiles.
              (fp16 operands stream at HALF rate on the PE - use bf16.)
  round(DVE): k = (t + 1.5*2^23) - 1.5*2^23  (exact fp32 rint), bf16 out.
  corr (PE):  t -= I @ k accumulated into the same PSUM group -> s in
              [-0.5, 0.5] (exact Sterbenz subtraction).  negi is fp8e4
              (exact -1.0) so its LDWEIGHTS runs at FWL 4-byte rate;
              mixed fp8-stationary x bf16-moving matmuls work fine.
  sin  (ACT): phi = Sin(2pi * s) = cos(x.a + b), bf16 -> SBUF.
  m2   (PE):  cps[J][32g:32g+32] += wsc[:,c,:].T @ phi-half, col-group
              g = (c + 2h) % 4 per n-half h; wsc zero-padded to M=32 so
              every cps row the epilogue reads is written (NaN hygiene -
              unwritten PSUM can hold NaN and NaN*0 = NaN in the reduce).
  epilogue:   ACT copies cps -> SBUF bf16 stage; PE does transpose AND
              4-way col-group reduction in one matmul per 128-n block
              (lhsT = stage slice, rhs = SEL selector); DVE scales the
              [128,128] result by 1/W_PRESCALE; DMA to DRAM.

x is transposed/split/packed to bf16 on the host (free), so there is no
device prologue; only live rows (0:34 -> 0:34, 34:68 -> 64:98) ship.
W is prescaled by 256 to keep bf16 wsc away from tiny magnitudes.
PSUM: 3 t-tiles (12KB/part) + 1 cps slot (4KB/part, reused by the
epilogue's ps2) = exactly 16KB/part.

Notes from tuning: Sin is only valid on [-pi, pi] (wide args return
garbage); AluOpType.mod fails the tensor_scalar ISA check; composed
row+col tile_position (K-split m2) hangs the device; matmul output is
capped at one PSUM bank (512 fp32 cols); DVE fp8 OUTPUT conversion is
broken (zeros) but host-supplied fp8 stationary operands are fine;
splitting a matmul by output partitions (M) does NOT reduce its cost
(each piece still streams every moving column).  DVE round (1x from
PSUM fp32, ~1.19us/tile) and ACT sin (~1.08us/tile) are at hardware
floors; PE (~167-171us busy at 2.4GHz, 98% of its 3-streams-per-tile
floor) is the wall.  Wall-clock ~196us = ~10us NEFF+DMA startup +
~181us PE span + ~6-10us NEFF drain tail.
"""

import math

import numpy as np

import concourse.bass as bass
import concourse.tile as tile
from concourse import bacc, mybir
from concourse.bass_utils import run_bass_kernel_spmd

F32 = mybir.dt.float32
BF16 = mybir.dt.bfloat16
FP8 = mybir.dt.float8e4

N, D, F, M = 32768, 16, 4096, 16
NCORES = 8
NLOC = N // NCORES            # 4096 rows per core
FC = F // 128                 # 32 f-chunks of 128
NJ = NLOC // 1024             # 4 n-groups of 1024

MAGIC = float(np.float32(1.5 * 2 ** 23))
TWO_PI = float(2.0 * np.pi)
W_PRESCALE = 256.0            # keep wsc fp16 away from subnormals

M2_LAG = 6                    # m2 consumes phi 6 iterations behind m1
NT = FC * NJ                  # 128 tiles

_CACHE = {}


def build_nc():
    nc = bacc.Bacc(None, target_bir_lowering=False)

    xq_in = nc.dram_tensor("xq_in", [68, NLOC], BF16, kind="ExternalInput")
    aq_in = nc.dram_tensor("aq_in", [68, FC, 128], BF16, kind="ExternalInput")
    wsc_in = nc.dram_tensor("wsc_in", [128, FC, 2 * M], BF16, kind="ExternalInput")
    negi_in = nc.dram_tensor("negi_in", [128, 128], FP8, kind="ExternalInput")
    sel_in = nc.dram_tensor("sel_in", [112, 16], BF16, kind="ExternalInput")
    out_t = nc.dram_tensor("out", [NLOC, M], F32, kind="ExternalOutput")

    with tile.TileContext(nc) as tc:
        with (
            tc.tile_pool(name="const", bufs=1) as const,
            tc.tile_pool(name="kp", bufs=4) as kp,
            tc.tile_pool(name="php", bufs=12) as php,
            tc.tile_pool(name="sg", bufs=3) as sg,
            tc.tile_pool(name="ob", bufs=3) as ob,
            tc.tile_pool(name="pst", bufs=3, space="PSUM") as pst,
            tc.tile_pool(name="pcs", bufs=1, space="PSUM") as pcs,
        ):
            # ---------------- constants ----------------
            # DMA order: first-needed first (aq/xq leading pieces unblock
            # m1 tile 0; negi unblocks corr; wsc unblocks m2).
            xq = const.tile([128, NLOC], BF16, tag="xq")
            aq = const.tile([128, FC, 128], BF16, tag="aq")
            wsc = const.tile([128, FC, 2 * M], BF16, tag="wsc")
            negi = const.tile([128, 128], FP8, tag="negi")
            sel = const.tile([112, 16], BF16, tag="sel")
            def dma_xa(dst, src_):
                # only rows 0:34 / 64:98 of xq & aq are live; input is compact
                nc.sync.dma_start(out=dst[0:34], in_=src_[0:34])
                nc.sync.dma_start(out=dst[64:98], in_=src_[34:68])

            nc.sync.dma_start(out=xq[0:34, 0:512], in_=xq_in[0:34, 0:512])
            nc.sync.dma_start(out=aq[0:34, 0:1, :], in_=aq_in[0:34, 0:1, :])
            nc.sync.dma_start(out=xq[64:98, 0:512], in_=xq_in[34:68, 0:512])
            nc.sync.dma_start(out=aq[64:98, 0:1, :], in_=aq_in[34:68, 0:1, :])
            dma_xa(xq[:, 512:1024], xq_in[:, 512:1024])
            nc.sync.dma_start(out=negi, in_=negi_in[:])
            dma_xa(aq[:, 1:4, :], aq_in[:, 1:4, :])
            nc.sync.dma_start(out=wsc, in_=wsc_in[:])
            nc.sync.dma_start(out=sel, in_=sel_in[:])
            for p in range(4, FC, 4):
                dma_xa(aq[:, p:p + 4, :], aq_in[:, p:p + 4, :])
            for j in range(1, NJ):
                dma_xa(xq[:, 1024 * j:1024 * (j + 1)],
                       xq_in[:, 1024 * j:1024 * (j + 1)])

            # Preload the Sin ACT table during the DMA wait.
            dummy = const.tile([1, 8], F32, tag="dummy")
            nc.gpsimd.memset(dummy, 0.25)
            dummy2 = const.tile([1, 8], BF16, tag="dummy2")
            nc.scalar.activation(out=dummy2, in_=dummy,
                                 func=mybir.ActivationFunctionType.Sin,
                                 bias=0.0, scale=1.0)

            # ---------------- main loop (software-pipelined) ----------------
            t_tiles = {}
            k_tiles = {}
            phi_tiles = {}
            cps_by_j = {}

            def emit_epilogue(j):
                cps = cps_by_j.pop(j)
                stage = sg.tile([112, 1024], BF16, tag="stage")
                nc.scalar.copy(out=stage, in_=cps[0:112, :])
                ps2 = pcs.tile([128, 1024], F32, tag="cps")
                for qq in range(8):
                    nc.tensor.matmul(
                        ps2[:, 16 * qq:16 * (qq + 1)],
                        stage[:, 128 * qq:128 * (qq + 1)],
                        sel,
                        start=True, stop=True,
                    )
                obuf = ob.tile([128, 128], F32, tag="obuf")
                for half in range(2):
                    nc.vector.tensor_scalar(
                        out=obuf[:, 64 * half:64 * (half + 1)],
                        in0=ps2[:, 64 * half:64 * (half + 1)],
                        scalar1=1.0 / W_PRESCALE, scalar2=None,
                        op0=mybir.AluOpType.mult,
                    )
                    nc.sync.dma_start(
                        out=out_t[1024 * j + 512 * half:
                                  1024 * j + 512 * (half + 1), :].rearrange(
                            "(qq p) m -> p qq m", qq=4
                        ),
                        in_=obuf[:, 64 * half:64 * (half + 1)].rearrange(
                            "p (qq m) -> p qq m", qq=4
                        ),
                    )

            for it in range(NT + M2_LAG + 1):
                # ---- m1(it) ----
                if it < NT:
                    j, c = divmod(it, FC)
                    tp = pst.tile([128, 1024], F32, tag="t")
                    for h in range(2):
                        grp = 64 * h
                        nc.tensor.matmul(
                            tp[:, 512 * h:512 * (h + 1)],
                            aq[grp:grp + 34, c, :],
                            xq[grp:grp + 34,
                               1024 * j + 512 * h:1024 * j + 512 * (h + 1)],
                            start=True, stop=False,
                            tile_position=(grp, 0),
                        )
                    t_tiles[it] = tp
                # ---- round(it-1) ----
                if 0 <= it - 1 < NT:
                    tp = t_tiles[it - 1]
                    k = kp.tile([128, 1024], BF16, tag="k")
                    nc.vector.tensor_scalar(
                        out=k, in0=tp,
                        scalar1=MAGIC, scalar2=MAGIC,
                        op0=mybir.AluOpType.add, op1=mybir.AluOpType.subtract,
                    )
                    k_tiles[it - 1] = k
                # ---- m2(it-M2_LAG) ----
                if 0 <= it - M2_LAG < NT:
                    it6 = it - M2_LAG
                    j6, c6 = divmod(it6, FC)
                    if c6 == 0:
                        cps_by_j[j6] = pcs.tile([128, 1024], F32, tag="cps", name="cps")
                    phi = phi_tiles.pop(it6)
                    for h in range(2):
                        gh = (c6 + 2 * h) % 4
                        nc.tensor.matmul(
                            cps_by_j[j6][32 * gh:32 * gh + 32,
                                         512 * h:512 * (h + 1)],
                            wsc[:, c6, :],
                            phi[:, 512 * h:512 * (h + 1)],
                            start=(c6 < 4), stop=(c6 >= 28),
                            tile_position=(0, 32 * gh),
                        )
                    if c6 == FC - 1:
                        emit_epilogue(j6)

                # ---- corr(it-2) + sin(it-2) ----
                if 0 <= it - 2 < NT:
                    tp = t_tiles.pop(it - 2)
                    k = k_tiles.pop(it - 2)
                    for h in range(2):
                        nc.tensor.matmul(
                            tp[:, 512 * h:512 * (h + 1)], negi,
                            k[:, 512 * h:512 * (h + 1)],
                            start=False, stop=True,
                        )
                    phi = php.tile([128, 1024], BF16, tag="phi")
                    nc.scalar.activation(
                        out=phi, in_=tp,
                        func=mybir.ActivationFunctionType.Sin,
                        bias=0.0, scale=TWO_PI,
                    )
                    phi_tiles[it - 2] = phi
    nc.finalize()
    return nc


def _host_prep(a, b, W):
    """Precompute replicated bf16 operand packs (float64 for exact splits)."""
    import ml_dtypes
    bf16 = ml_dtypes.bfloat16
    inv2pi = 1.0 / (2.0 * np.pi)
    a64 = np.asarray(a, dtype=np.float64).T * inv2pi          # [16, F]
    b64 = (np.asarray(b, dtype=np.float64) + np.pi / 2.0) * inv2pi  # [F]
    ah = a64.astype(bf16)                                      # single limb
    bh = b64.astype(bf16)
    bl = (b64 - bh.astype(np.float64)).astype(bf16)

    # K=34 rows: [ah (vs xh); ah (vs xl); bh; bl], duplicated in both
    # 64-row-groups so a tile's two 512-halves run as concurrent row tiles.
    aq = np.zeros((68, FC, 128), dtype=bf16)
    for c in range(FC):
        sl = slice(128 * c, 128 * (c + 1))
        for grp in (0, 34):
            aq[grp:grp + 16, c, :] = ah[:, sl]
            aq[grp + 16:grp + 32, c, :] = ah[:, sl]
            aq[grp + 32, c, :] = bh[sl]
            aq[grp + 33, c, :] = bl[sl]

    scale = math.sqrt(2.0 / F) * W_PRESCALE
    W2 = (np.asarray(W, dtype=np.float64).reshape(F, M) * scale).astype(bf16)
    wsc = np.zeros((128, FC, 2 * M), dtype=bf16)               # zero-padded M
    wsc[:, :, 0:M] = W2.reshape(FC, 128, M).transpose(1, 0, 2)

    negi = (-np.eye(128)).astype(ml_dtypes.float8_e4m3fn)
    sel = np.zeros((112, 16), dtype=bf16)
    for g in range(4):
        for m in range(16):
            sel[32 * g + m, m] = 1.0
    return aq, wsc, negi, sel


def _pack_x(xs):
    """xs [NLOC, D] fp32 -> xq [128, NLOC] bf16 (xh/xl/ones, dup at row 64)."""
    import ml_dtypes
    bf16 = ml_dtypes.bfloat16
    x64 = np.asarray(xs, dtype=np.float64).T                   # [16, NLOC]
    xh = x64.astype(bf16)
    xl = (x64 - xh.astype(np.float64)).astype(bf16)
    xq = np.zeros((68, NLOC), dtype=bf16)
    for grp in (0, 34):
        xq[grp:grp + 16, :] = xh
        xq[grp + 16:grp + 32, :] = xl
        xq[grp + 32:grp + 34, :] = bf16(1.0)
    return xq


def make_in_maps(x, a, b, W):
    x = np.ascontiguousarray(np.asarray(x, dtype=np.float32))
    aq, wsc, negi, sel = _host_prep(a, b, W)
    in_maps = []
    for i in range(NCORES):
        in_maps.append({
            "xq_in": _pack_x(x[i * NLOC:(i + 1) * NLOC]),
            "aq_in": aq,
            "wsc_in": wsc,
            "negi_in": negi,
            "sel_in": sel,
        })
    return in_maps


def kernel(x, a, b, W):
    if "nc" not in _CACHE:
        _CACHE["nc"] = build_nc()
    nc = _CACHE["nc"]
    in_maps = make_in_maps(x, a, b, W)
    res = run_bass_kernel_spmd(nc, in_maps, core_ids=list(range(NCORES)))
    return np.concatenate([r["out"] for r in res.results], axis=0)

